# revision 1
# baseline (speedup 1.0000x reference)
"""BankedLinear (MoE-style banked linear) Trainium2 Bass kernel.

Math: out[n] = sum_k bank_weights[n,k] * (tensor[n] @ W[sel[n,k]] + bias[sel[n,k]])
Shapes: tensor [8192,128] f32, bank_weights [8192,2] f32, bank_selections [8192,2] int,
        weights [64,128,128] f32, bias [64,128] f32 -> out [8192,128] f32.

Strategy (data parallel over tokens, weights replicated):
  - 8 cores x 1024 tokens. The host computes routing metadata only: a
    load-balanced token->core assignment, the sort of each core's 2048
    (token,k) pairs by bank id (gather/unpermute index arrays), a bank
    capacity plan shared by all cores (SPMD: one program), and the routing
    matrix pt[b,n] = sum_k bw[n,k]*[sel[n,k]==b] used for the bias term.
  - On device per core:
      1. dma_gather sorted token rows from DRAM x -> SBUF tiles [128,128]
      2. PE-transpose each tile -> Xs^T [128(in), Ctot] in SBUF
      3. per bank b: matmul(psum[:, seg] = W_b^T @ Xs^T[:, seg]) (fp32);
         weights stream in three slices over the ACT/SP/Pool DMA paths in
         bank-processing order so early banks start as soon as possible
      4. copy psum -> Y^T SBUF [128(out), Ctot]
      5. PE-transpose Y^T back to row layout, scaling rows by the sorted
         bank_weights during the PSUM->SBUF copy; quartered DMA to scratch Y
      6. two token-half pipelines: fused dma_gather of Y rows by inverse
         permutation (k=0 and k=1), out = g0 + g1 + b_tok where
         b_tok = pt^T @ bias was computed on the PE during phase 3.
"""

import numpy as np

N, K, IN, OUT, NUM_BANKS = 8192, 2, 128, 128, 64
NCORES = 8
NLOC = N // NCORES  # tokens per core
P = 128
PSUM_FREE = 512  # max fp32 matmul moving free dim / psum bank
W_SPLITS = (16, 36, 12)  # banks per DMA path: ACT, SP, Pool (in bank order)


def _routing_plan(sel_all):
    """sel_all: [N, K] int. Balances tokens across cores to minimize per-bank
    capacity (max over cores), then builds per-core routing index arrays.
    Returns (assign [NCORES, NLOC] token ids, caps, offs, Ctot, per_core)."""
    sel_all = np.asarray(sel_all).astype(np.int64)
    gcount = np.bincount(sel_all.reshape(-1), minlength=NUM_BANKS)
    ideal = (gcount + NCORES - 1) // NCORES  # per-core target per bank
    counts = np.zeros((NCORES, NUM_BANKS), dtype=np.int64)
    fill = np.zeros(NCORES, dtype=np.int64)
    assign_lists = [[] for _ in range(NCORES)]
    for n in range(N):
        b0, b1 = int(sel_all[n, 0]), int(sel_all[n, 1])
        best, best_key = -1, None
        for c in range(NCORES):
            if fill[c] >= NLOC:
                continue
            over = max(0, counts[c, b0] + 1 - ideal[b0])
            if b1 == b0:
                over += max(0, counts[c, b0] + 2 - ideal[b0])
            else:
                over += max(0, counts[c, b1] + 1 - ideal[b1])
            key = (over, counts[c, b0] + counts[c, b1], fill[c])
            if best < 0 or key < best_key:
                best, best_key = c, key
        counts[best, b0] += 1
        counts[best, b1] += 1
        fill[best] += 1
        assign_lists[best].append(n)
    assign = np.array(assign_lists, dtype=np.int64)  # [NCORES, NLOC]

    caps = counts.max(axis=0).astype(np.int64)
    pad = (-int(caps.sum())) % P
    for i in range(pad):
        caps[i % NUM_BANKS] += 1
    Ctot = int(caps.sum())
    offs = np.concatenate([[0], np.cumsum(caps)[:-1]]).astype(np.int64)

    per_core = []
    for c in range(NCORES):
        sel = sel_all[assign[c]]                 # [NLOC, K]
        gidx = np.zeros(Ctot, dtype=np.int16)    # sorted-slot -> local token row
        inv = np.zeros((NLOC, K), dtype=np.int16)  # (token,k) -> sorted slot
        fillb = offs.copy()
        for i in range(NLOC):
            for k in range(K):
                b = sel[i, k]
                slot = fillb[b]
                fillb[b] += 1
                gidx[slot] = i
                inv[i, k] = slot
        per_core.append((gidx, inv))
    return assign, caps, offs, Ctot, per_core


def _wrap_idx(flat_idx):
    """Wrap a flat int16 index list into the [128, n//16] SWDGE layout:
    index i lives at [i % 16, i // 16], replicated across the 8 Q7 groups."""
    n = flat_idx.shape[0]
    assert n % 16 == 0
    w = flat_idx.reshape(n // 16, 16).T.astype(np.int16)  # [16, n//16]
    return np.tile(w, (8, 1))  # [128, n//16]


def _build_program(caps, offs, Ctot):
    import concourse.bacc as bacc
    import concourse.tile as tile
    from concourse import mybir, library_config
    from concourse.masks import make_identity
    from concourse.tile import add_dep_helper

    f32 = mybir.dt.float32
    i16 = mybir.dt.int16

    nblk = Ctot // P
    ntok_blk = NLOC // P
    nsplit = [0] + list(np.cumsum(W_SPLITS))  # bank boundaries of the 3 slices

    nc = bacc.Bacc(None, target_bir_lowering=False, debug=False)

    x_d = nc.declare_dram_parameter("x", [NLOC, IN], f32, isOutput=False)
    w_d = nc.declare_dram_parameter("wts", [NUM_BANKS, IN, OUT], f32, isOutput=False)
    bias_d = nc.declare_dram_parameter("biasb", [NUM_BANKS, OUT], f32, isOutput=False)
    pt_d = nc.declare_dram_parameter("ptmat", [NUM_BANKS, NLOC], f32, isOutput=False)
    bws_d = nc.declare_dram_parameter("bws", [Ctot, 1], f32, isOutput=False)
    gidx_d = nc.declare_dram_parameter("gidx", [P, Ctot // 16], i16, isOutput=False)
    ginv_d = nc.declare_dram_parameter("ginv", [P, (2 * NLOC) // 16], i16,
                                       isOutput=False)
    out_d = nc.declare_dram_parameter("out", [NLOC, OUT], f32, isOutput=True)
    y_d = nc.dram_tensor("yscratch", [Ctot, OUT], f32)

    # psum column groups: per-bank column chunks (<=512 each for the psum
    # bank limit) packed into <=512-wide psum tiles
    chunks = []  # (bank, col_start, width)
    for b in range(NUM_BANKS):
        cb, ob = int(caps[b]), int(offs[b])
        while cb > 0:
            w = min(cb, PSUM_FREE)
            chunks.append((b, ob, w))
            ob += w
            cb -= w
    groups = []  # (col_start, width, [(bank, seg_off_in_group, cb)])
    cur = None
    for (b, ob, cb) in chunks:
        if cur is not None and (ob + cb - cur[0]) <= PSUM_FREE:
            cur[2].append((b, ob - cur[0], cb))
            cur[1] = ob + cb - cur[0]
        else:
            if cur is not None:
                groups.append(tuple(cur))
            cur = [ob, cb, [(b, 0, cb)]]
    groups.append(tuple(cur))

    with tile.TileContext(nc) as tc:
        with (
            tc.tile_pool(name="const", bufs=1) as cpool,
            tc.tile_pool(name="big", bufs=1) as bigpool,
            tc.tile_pool(name="psum_t", bufs=4, space="PSUM") as psum_t,
            tc.tile_pool(name="psum_y", bufs=3, space="PSUM") as psum_y,
            tc.tile_pool(name="psum_b", bufs=1, space="PSUM") as psum_b,
        ):
            ident = cpool.tile([P, P], f32)
            make_identity(nc, ident[:])
            # prime the ACT Copy LUT while DMAs run so the first real
            # activation op doesn't pay the table load mid-pipeline
            warm = cpool.tile([P, 1], f32)
            nc.vector.memset(warm[:], 0.0)
            nc.scalar.activation(warm[:], warm[:],
                                 mybir.ActivationFunctionType.Copy)

            gidx_sb = cpool.tile([P, Ctot // 16], i16)
            nc.sync.dma_start(out=gidx_sb[:], in_=gidx_d.ap())
            libload = nc.gpsimd.load_library(library_config.mlp)

            # Phase A: gather sorted token rows (split for earlier transposes)
            xg = bigpool.tile([P, nblk, IN], f32, tag="xg")
            halfblk = nblk // 2
            ga = nc.gpsimd.dma_gather(
                out_ap=xg[:, :halfblk, :], in_ap=x_d.ap(),
                idxs_ap=gidx_sb[:, :halfblk * 8],
                num_idxs=halfblk * P, num_idxs_reg=halfblk * P, elem_size=IN,
                single_packet=halfblk * P <= 1024,
            )
            gb = nc.gpsimd.dma_gather(
                out_ap=xg[:, halfblk:, :], in_ap=x_d.ap(),
                idxs_ap=gidx_sb[:, halfblk * 8:],
                num_idxs=(nblk - halfblk) * P, num_idxs_reg=(nblk - halfblk) * P,
                elem_size=IN, single_packet=(nblk - halfblk) * P <= 1024,
            )
            add_dep_helper(ga.ins, libload.ins, sync=False,
                           reason="gather needs mlp gpsimd library")
            add_dep_helper(gb.ins, libload.ins, sync=False,
                           reason="gather needs mlp gpsimd library")

            # weights in three bank slices: ACT ring, SP ring, Pool (SWDGE)
            w_parts = []
            for si, eng in zip(range(3), (nc.scalar, nc.sync, nc.gpsimd)):
                b0, b1 = nsplit[si], nsplit[si + 1]
                wp = bigpool.tile([P, (b1 - b0) * OUT], f32, tag=f"w{si}")
                wdma = eng.dma_start(
                    out=wp[:].rearrange("i (b o) -> i b o", o=OUT),
                    in_=w_d[b0:b1].rearrange("b i o -> i b o"),
                )
                if eng is nc.gpsimd:
                    add_dep_helper(wdma.ins, ga.ins, sync=False,
                                   reason="pool weight slice waits on x gathers")
                    add_dep_helper(wdma.ins, gb.ins, sync=False,
                                   reason="pool weight slice waits on x gathers")
                w_parts.append(wp)

            def w_slice(b):
                for si in range(3):
                    if nsplit[si] <= b < nsplit[si + 1]:
                        lo = (b - nsplit[si]) * OUT
                        return w_parts[si][:, lo:lo + OUT]
                raise AssertionError(b)

            # small loads on the SP ring after its weight slice
            ginv_sb = cpool.tile([P, (2 * NLOC) // 16], i16)
            nc.sync.dma_start(out=ginv_sb[:], in_=ginv_d.ap())
            bws_sb = cpool.tile([P, nblk, 1], f32)
            nc.sync.dma_start(out=bws_sb[:],
                              in_=bws_d.ap().rearrange("(t p) o -> p t o", p=P))
            bias_sb = cpool.tile([NUM_BANKS, OUT], f32)
            nc.sync.dma_start(out=bias_sb[:], in_=bias_d.ap())
            pt_sb = cpool.tile([NUM_BANKS, NLOC], f32)
            nc.sync.dma_start(out=pt_sb[:], in_=pt_d.ap())

            # Xs^T via PE transposes
            xsT = bigpool.tile([P, Ctot], f32, tag="xsT")
            for t in range(nblk):
                ptt = psum_t.tile([P, P], f32, tag="ptt")
                nc.tensor.transpose(out=ptt[:], in_=xg[:, t, :], identity=ident[:])
                if t % 2 == 0:
                    nc.vector.tensor_copy(xsT[:, t * P:(t + 1) * P], ptt[:])
                else:
                    nc.scalar.copy(xsT[:, t * P:(t + 1) * P], ptt[:])

            # bias-term matmuls (pt^T @ bias), early, parked in SBUF
            b_tok = bigpool.tile([P, ntok_blk, OUT], f32, tag="b_tok")
            for j in range(ntok_blk):
                pb = psum_b.tile([P, OUT], f32, tag="pb")
                nc.tensor.matmul(out=pb[:], lhsT=pt_sb[:, j * P:(j + 1) * P],
                                 rhs=bias_sb[:], start=True, stop=True)
                if j % 2 == 0:
                    nc.scalar.copy(b_tok[:, j, :], pb[:])
                else:
                    nc.vector.tensor_copy(b_tok[:, j, :], pb[:])

            # Phase B/C: per-bank matmuls into packed psum tiles, copy to Y^T
            ysT = bigpool.tile([P, Ctot], f32, tag="ysT")
            for gi, (col0, width, banks) in enumerate(groups):
                py = psum_y.tile([P, PSUM_FREE], f32, tag="py")
                for (b, so, cb) in banks:
                    nc.tensor.matmul(
                        out=py[:, so:so + cb],
                        lhsT=w_slice(b),
                        rhs=xsT[:, col0 + so: col0 + so + cb],
                        start=True, stop=True,
                    )
                h = width // 2
                if h > 0:
                    nc.vector.tensor_copy(ysT[:, col0:col0 + h], py[:, :h])
                    nc.scalar.copy(ysT[:, col0 + h:col0 + width], py[:, h:width])
                else:
                    nc.vector.tensor_copy(ysT[:, col0:col0 + width], py[:, :width])

            # Phase D: transpose Y^T back to row layout, scale rows by sorted
            # bank_weights during the PSUM->SBUF copy, quartered stores
            yrows = bigpool.tile([P, nblk, OUT], f32, tag="yrows")
            for t in range(nblk):
                ptt = psum_t.tile([P, P], f32, tag="ptt")
                nc.tensor.transpose(out=ptt[:], in_=ysT[:, t * P:(t + 1) * P],
                                    identity=ident[:])
                if t % 2 == 0:
                    nc.vector.tensor_scalar_mul(yrows[:, t, :], ptt[:],
                                                bws_sb[:, t, 0:1])
                else:
                    nc.scalar.activation(yrows[:, t, :], ptt[:],
                                         mybir.ActivationFunctionType.Copy,
                                         scale=bws_sb[:, t, 0:1])
            qb = [0, nblk // 4, nblk // 2, (3 * nblk) // 4, nblk]
            for qi in range(4):
                t0q, t1q = qb[qi], qb[qi + 1]
                eng = nc.sync if qi % 2 == 0 else nc.gpsimd
                eng.dma_start(
                    out=y_d[t0q * P:t1q * P].rearrange("(t p) o -> p t o", p=P),
                    in_=yrows[:, t0q:t1q, :])

            # Phase E: two token-half pipelines of gather -> adds -> store
            htok = ntok_blk // 2
            o_all = bigpool.tile([P, ntok_blk, OUT], f32, tag="o_all")
            for hi in range(2):
                g01 = bigpool.tile([P, ntok_blk, OUT], f32, tag=f"g01_{hi}")
                ge = nc.gpsimd.dma_gather(
                    out_ap=g01[:], in_ap=y_d.ap(),
                    idxs_ap=ginv_sb[:, hi * (NLOC // 16):(hi + 1) * (NLOC // 16)],
                    num_idxs=NLOC, num_idxs_reg=NLOC, elem_size=OUT,
                    single_packet=NLOC <= 1024,
                )
                add_dep_helper(ge.ins, libload.ins, sync=False,
                               reason="gather needs mlp gpsimd library")
                ja, jb = hi * htok, (hi + 1) * htok
                nc.vector.tensor_add(out=o_all[:, ja:jb, :],
                                     in0=g01[:, :htok, :], in1=g01[:, htok:, :])
                nc.vector.tensor_add(out=o_all[:, ja:jb, :],
                                     in0=o_all[:, ja:jb, :],
                                     in1=b_tok[:, ja:jb, :])
                eng = nc.sync if hi == 0 else nc.gpsimd
                eng.dma_start(
                    out=out_d[ja * P:jb * P].rearrange("(j p) o -> p j o", p=P),
                    in_=o_all[:, ja:jb, :])

    return nc


def _make_in_maps(tensor, bank_weights, bank_selections, bias, weights,
                  assign, caps, offs, Ctot, per_core):
    tensor = np.ascontiguousarray(tensor, dtype=np.float32)
    bank_weights = np.ascontiguousarray(bank_weights, dtype=np.float32)
    sel_all = np.asarray(bank_selections).astype(np.int64)
    weights = np.ascontiguousarray(weights, dtype=np.float32)
    bias_bf = np.ascontiguousarray(bias, dtype=np.float32)
    in_maps = []
    ntok_half = NLOC // 2
    for c in range(NCORES):
        gidx, inv = per_core[c]
        toks = assign[c]
        bw = bank_weights[toks]                             # [NLOC, K]
        sel = sel_all[toks]                                 # [NLOC, K]
        # sorted bank weights: bws[slot] = bw of the pair at that slot (0 pad)
        bws = np.zeros((Ctot, 1), dtype=np.float32)
        bws[inv.reshape(-1).astype(np.int64), 0] = bw.reshape(-1)
        # routing matrix pt[b, n] = sum_k bw[n,k] * [sel[n,k]==b]
        ptm = np.zeros((NUM_BANKS, NLOC), dtype=np.float32)
        rows = sel.reshape(-1)
        cols = np.repeat(np.arange(NLOC, dtype=np.int64), K)
        np.add.at(ptm, (rows, cols), bw.reshape(-1))
        # gather-back index order: token halves, each with its k=0 then k=1 ids
        ginv = np.concatenate([inv[:ntok_half, 0], inv[:ntok_half, 1],
                               inv[ntok_half:, 0], inv[ntok_half:, 1]])
        in_maps.append({
            "x": np.ascontiguousarray(tensor[toks]),
            "wts": weights,
            "biasb": bias_bf,
            "ptmat": ptm,
            "bws": bws,
            "gidx": _wrap_idx(gidx),
            "ginv": _wrap_idx(ginv),
        })
    return in_maps


def kernel(tensor, bank_weights, bank_selections, weights, bias):
    tensor = np.asarray(tensor)
    bank_weights = np.asarray(bank_weights)
    bank_selections = np.asarray(bank_selections)
    weights = np.asarray(weights)
    bias = np.asarray(bias)

    assign, caps, offs, Ctot, per_core = _routing_plan(bank_selections)
    nc = _build_program(caps, offs, Ctot)
    in_maps = _make_in_maps(tensor, bank_weights, bank_selections, bias, weights,
                            assign, caps, offs, Ctot, per_core)

    nc.finalize()
    from concourse.bass_utils import run_bass_kernel_spmd
    try:
        res = run_bass_kernel_spmd(nc, in_maps, list(range(NCORES)))
    except Exception:
        # one retry: a previous crashed session can leave the accelerator in
        # a transient bad state that clears on the next dispatch
        import time
        time.sleep(2.0)
        res = run_bass_kernel_spmd(nc, in_maps, list(range(NCORES)))
    out = np.empty((N, OUT), dtype=np.float32)
    for c in range(NCORES):
        out[assign[c]] = res.results[c]["out"]
    return out



# revision 13
# speedup vs baseline: 1.1496x; 1.1496x over previous
"""BankedLinear (MoE-style banked linear) Trainium2 Bass kernel.

Math: out[n] = sum_k bank_weights[n,k] * (tensor[n] @ W[sel[n,k]] + bias[sel[n,k]])
Shapes: tensor [8192,128] f32, bank_weights [8192,2] f32, bank_selections [8192,2] int,
        weights [64,128,128] f32, bias [64,128] f32 -> out [8192,128] f32.

Strategy (data parallel over tokens, bf16 compute, two sorted passes):
  - 8 cores x 1024 tokens, greedy-balanced so per-bank per-pass counts are
    nearly equal across cores (SPMD: one program, shared bank capacities).
  - Pass A handles every token's k=0 pair sorted by sel0; pass B handles k=1
    sorted by sel1. Each pass:
      x slots loaded straight into transposed SBUF layout via dma_transpose
      (bf16), then per-bank matmuls W_b^T @ xT accumulate y^T into psum
      ([128 out, token cols], bf16 => 1 cycle/row). The bias term is seeded
      into the same psum by a rank-64 matmul bias^T @ H (H = 0/1 one-hot of
      the slot's bank, host built).
  - psum -> SBUF bf16 copy, PE transpose back to row layout, and the
    psum->SBUF copy of the transpose applies the per-token bank_weight as a
    tensor_scalar multiply (free).
  - Pass A rows are stored contiguously to the DRAM output; pass B rows are
    dma_scatter_add-ed (SWDGE, descriptors pre-generated via prepare_only)
    into the same buffer at the A-slot of the same token. Pad slots compute
    exact zeros and are pointed at row 0 (add of 0) or dropped by the host.
  - Host unpermutes: out[token_of_A_slot] = out_big[slot].
"""

import numpy as np
import ml_dtypes

N, K, IN, OUT, NUM_BANKS = 8192, 2, 128, 128, 64
NCORES = 8
NLOC = N // NCORES  # tokens per core
P = 128
PSUM_FREE = 512  # fp32 columns per psum bank
BF16 = ml_dtypes.bfloat16


def _routing_plan(sel_all):
    """Balance tokens across cores so that per-core per-bank counts of sel0
    and sel1 are close to the global ideal. Returns (assign [NCORES, NLOC],
    caps0, caps1) with caps shared by all cores (SPMD program)."""
    sel_all = np.asarray(sel_all).astype(np.int64)
    g0 = np.bincount(sel_all[:, 0], minlength=NUM_BANKS)
    g1 = np.bincount(sel_all[:, 1], minlength=NUM_BANKS)
    ideal0 = (g0 + NCORES - 1) // NCORES
    ideal1 = (g1 + NCORES - 1) // NCORES
    c0 = np.zeros((NCORES, NUM_BANKS), dtype=np.int64)
    c1 = np.zeros((NCORES, NUM_BANKS), dtype=np.int64)
    fill = np.zeros(NCORES, dtype=np.int64)
    assign_lists = [[] for _ in range(NCORES)]
    for n in range(N):
        b0, b1 = int(sel_all[n, 0]), int(sel_all[n, 1])
        best, best_key = -1, None
        for c in range(NCORES):
            if fill[c] >= NLOC:
                continue
            over = max(0, c0[c, b0] + 1 - ideal0[b0]) + max(
                0, c1[c, b1] + 1 - ideal1[b1])
            key = (over, c0[c, b0] + c1[c, b1], fill[c])
            if best < 0 or key < best_key:
                best, best_key = c, key
        c0[best, b0] += 1
        c1[best, b1] += 1
        fill[best] += 1
        assign_lists[best].append(n)
    assign = np.array(assign_lists, dtype=np.int64)

    caps0 = c0.max(axis=0).astype(np.int64)
    caps1 = c1.max(axis=0).astype(np.int64)
    # pad total slot counts to a multiple of 128 by growing the last bank
    caps0[NUM_BANKS - 1] += (-int(caps0.sum())) % P
    caps1[NUM_BANKS - 1] += (-int(caps1.sum())) % P
    return assign, caps0, caps1


def _offsets(caps):
    return np.concatenate([[0], np.cumsum(caps)[:-1]]).astype(np.int64)


def _segments(caps, offs):
    """Shared matmul segment list: (psum_tile, col0_in_tile, width, bank),
    bank ranges split at psum-tile (512 col) boundaries."""
    segs = []
    for b in range(NUM_BANKS):
        if caps[b] == 0:
            continue
        s0, s1 = int(offs[b]), int(offs[b] + caps[b])
        while s0 < s1:
            ti = s0 // PSUM_FREE
            e = min(s1, (ti + 1) * PSUM_FREE)
            segs.append((ti, s0 - ti * PSUM_FREE, e - s0, b))
            s0 = e
    return segs


def _wrap_idx(flat_idx):
    """Wrap a flat int16 index list into the [128, n//16] SWDGE layout:
    index i lives at [i % 16, i // 16], replicated across the 8 Q7 groups."""
    n = flat_idx.shape[0]
    assert n % 16 == 0
    w = flat_idx.reshape(n // 16, 16).T.astype(np.int16)
    return np.tile(w, (8, 1))


def _build_program(C0, C1, segsA, segsB):
    import concourse.bacc as bacc
    import concourse.tile as tile
    from concourse import mybir, library_config
    from concourse.masks import make_identity
    from concourse.tile import add_dep_helper

    f32 = mybir.dt.float32
    bf16 = mybir.dt.bfloat16
    i16 = mybir.dt.int16

    nbA, nbB = C0 // P, C1 // P  # 128-row blocks per pass
    tilesA = [min(PSUM_FREE, C0 - t) for t in range(0, C0, PSUM_FREE)]
    tilesB = [min(PSUM_FREE, C1 - t) for t in range(0, C1, PSUM_FREE)]
    assert len(tilesA) <= 3 and len(tilesB) <= 3, (C0, C1)

    nc = bacc.Bacc(None, target_bir_lowering=False, debug=False)

    xA_d = nc.declare_dram_parameter("xa", [C0, IN], bf16, isOutput=False)
    xB_d = nc.declare_dram_parameter("xb", [C1, IN], bf16, isOutput=False)
    w_d = nc.declare_dram_parameter("wts", [IN, NUM_BANKS * OUT], bf16,
                                    isOutput=False)
    bias_d = nc.declare_dram_parameter("biasb", [NUM_BANKS, OUT], bf16,
                                       isOutput=False)
    h_d = nc.declare_dram_parameter("h01", [NUM_BANKS, C0 + C1], bf16,
                                    isOutput=False)
    bw_d = nc.declare_dram_parameter("bwab", [P, (C0 + C1) // P], f32,
                                     isOutput=False)
    sidx_d = nc.declare_dram_parameter("sidx", [P, C1 // 16], i16,
                                       isOutput=False)
    # extra dump rows: every pad slot of pass B scatter-adds into its own
    # private row (concurrent adds to a shared row race on real SWDGE)
    npad = C1 - NLOC
    out_d = nc.declare_dram_parameter("out", [C0 + npad, OUT], f32,
                                      isOutput=True)

    with tile.TileContext(nc) as tc:
        with (
            tc.tile_pool(name="const", bufs=1) as cpool,
            tc.tile_pool(name="big", bufs=1) as bigpool,
            tc.tile_pool(name="psum_a", bufs=1, space="PSUM") as psum_a,
            tc.tile_pool(name="psum_b", bufs=1, space="PSUM") as psum_b,
            tc.tile_pool(name="psum_t", bufs=2, space="PSUM") as psum_t,
        ):
            ident = cpool.tile([P, P], bf16)
            make_identity(nc, ident[:])
            # prime the ACT Copy LUT so the first real activation op doesn't
            # pay the table load mid-pipeline
            warm = cpool.tile([P, 1], f32)
            nc.vector.memset(warm[:], 0.0)
            nc.scalar.activation(warm[:], warm[:],
                                 mybir.ActivationFunctionType.Copy)
            libload = nc.gpsimd.load_library(library_config.mlp)

            # small loads (DVE/ACT rings)
            bias_sb = cpool.tile([NUM_BANKS, OUT], bf16)
            nc.scalar.dma_start(out=bias_sb[:], in_=bias_d.ap())
            h_sb = cpool.tile([NUM_BANKS, C0 + C1], bf16)
            nc.scalar.dma_start(out=h_sb[:], in_=h_d.ap())
            bw_sb = cpool.tile([P, (C0 + C1) // P], f32)
            nc.scalar.dma_start(out=bw_sb[:], in_=bw_d.ap())
            sidx_sb = cpool.tile([P, C1 // 16], i16)
            nc.scalar.dma_start(out=sidx_sb[:], in_=sidx_d.ap())

            # x slots straight into transposed layout (xbar DMA transpose)
            xTA = bigpool.tile([P, C0], bf16, tag="xTA")
            nc.sync.dma_start_transpose(xTA[:], xA_d.ap())
            xTB = bigpool.tile([P, C1], bf16, tag="xTB")
            nc.sync.dma_start_transpose(xTB[:], xB_d.ap())

            # weights [in, bank*out] in two halves (banks 0-31, 32-63)
            wT = bigpool.tile([P, NUM_BANKS * OUT], bf16, tag="wT")
            half = (NUM_BANKS // 2) * OUT
            nc.sync.dma_start(out=wT[:, :half], in_=w_d[:, :half])
            nc.sync.dma_start(out=wT[:, half:], in_=w_d[:, half:])

            # psum tiles per pass
            psA = [psum_a.tile([P, w], f32, tag=f"a{i}", name=f"psa{i}")
                   for i, w in enumerate(tilesA)]
            psB = [psum_b.tile([P, w], f32, tag=f"b{i}", name=f"psb{i}")
                   for i, w in enumerate(tilesB)]

            def run_pass(ps, tiles, segs, h_off, ti):
                """Per-segment bias seed + weight matmul for psum tile ti.
                Seed: psum[o, i] = sum_b bias[b, o] * H[b, i], then the
                bank's W matmul accumulates on top of it."""
                xT = xTA if h_off == 0 else xTB
                for (t, c0, cw, b) in segs:
                    if t != ti:
                        continue
                    g0 = t * PSUM_FREE + c0
                    nc.tensor.matmul(
                        out=ps[ti][:, c0:c0 + cw],
                        lhsT=bias_sb[:],
                        rhs=h_sb[:, h_off + g0:h_off + g0 + cw],
                        start=True, stop=False,
                    )
                    nc.tensor.matmul(
                        out=ps[ti][:, c0:c0 + cw],
                        lhsT=wT[:, b * OUT:(b + 1) * OUT],
                        rhs=xT[:, g0:g0 + cw],
                        start=False, stop=True,
                    )

            # W1-era tiles first, then W2-era (PE SEQ is in order)
            tile_order = []
            for i in range(max(len(tilesA), len(tilesB))):
                if i < len(tilesA):
                    tile_order.append(("A", i))
                if i < len(tilesB):
                    tile_order.append(("B", i))
            for (side, i) in tile_order:
                if side == "A":
                    run_pass(psA, tilesA, segsA, 0, i)
                else:
                    run_pass(psB, tilesB, segsB, C0, i)

            # psum^T -> SBUF bf16, PE transpose back, scaled copy to rows
            aT = bigpool.tile([P, C0], bf16, tag="aT")
            bT = bigpool.tile([P, C1], bf16, tag="bT")
            o0 = bigpool.tile([P, nbA, OUT], f32, tag="o0")
            o1 = bigpool.tile([P, nbB, OUT], f32, tag="o1")

            eng_i = 0

            def copy_eng():
                nonlocal eng_i
                eng_i += 1
                return nc.vector if eng_i % 2 == 0 else nc.scalar

            def finish_tile(side, ti):
                tw = (tilesA if side == "A" else tilesB)[ti]
                ps = (psA if side == "A" else psB)[ti]
                yT = aT if side == "A" else bT
                orows = o0 if side == "A" else o1
                bw_off = 0 if side == "A" else nbA
                col0 = ti * PSUM_FREE
                # big psum -> SBUF bf16 copy
                e = copy_eng()
                if e is nc.vector:
                    nc.vector.tensor_copy(yT[:, col0:col0 + tw], ps[:, :])
                else:
                    nc.scalar.copy(yT[:, col0:col0 + tw], ps[:, :])
                # per 128-col block: PE transpose back + scaled copy
                for j0 in range(0, tw, P):
                    j = (col0 + j0) // P
                    pt = psum_t.tile([P, P], bf16, tag="pt")
                    nc.tensor.transpose(out=pt[:], in_=yT[:, col0 + j0:
                                                          col0 + j0 + P],
                                        identity=ident[:])
                    e = copy_eng()
                    if e is nc.vector:
                        nc.vector.tensor_scalar_mul(
                            orows[:, j, :], pt[:], bw_sb[:, bw_off + j:
                                                         bw_off + j + 1])
                    else:
                        nc.scalar.activation(
                            orows[:, j, :], pt[:],
                            mybir.ActivationFunctionType.Copy,
                            scale=bw_sb[:, bw_off + j:bw_off + j + 1])

            for (side, i) in tile_order:
                finish_tile(side, i)

            # pass A: contiguous store per psum tile
            blk0 = 0
            store_insts = []
            for i, tw in enumerate(tilesA):
                nb = tw // P
                st = nc.sync.dma_start(
                    out=out_d[blk0 * P:(blk0 + nb) * P].rearrange(
                        "(t p) o -> p t o", p=P),
                    in_=o0[:, blk0:blk0 + nb, :])
                store_insts.append(st)
                blk0 += nb

            # pass B: scatter-add per psum tile (prep early, trigger on ready)
            blk0 = 0
            for i, tw in enumerate(tilesB):
                nb = tw // P
                nidx = nb * P
                dma_sem = nc.alloc_semaphore(f"swdge_scat{i}")
                prep = nc.gpsimd.dma_scatter_add(
                    out_d.ap(),
                    o1[:, blk0:blk0 + nb, :],
                    sidx_sb[:, blk0 * 8:(blk0 + nb) * 8],
                    nidx, nidx, OUT,
                    prepare_only=True, sem=dma_sem,
                )
                add_dep_helper(prep.ins, libload.ins, sync=False,
                               reason="scatter needs mlp gpsimd library")
                trig = nc.gpsimd.trigger_dma(count=None)
                for st in store_insts:
                    add_dep_helper(trig.ins, st.ins, sync=True,
                                   reason="scatter adds into stored A rows")
                blk0 += nb

    return nc


def _make_in_maps(tensor, bank_weights, bank_selections, weights, bias,
                  assign, caps0, caps1):
    tensor = np.asarray(tensor, dtype=np.float32)
    bank_weights = np.asarray(bank_weights, dtype=np.float32)
    sel_all = np.asarray(bank_selections).astype(np.int64)
    offs0, offs1 = _offsets(caps0), _offsets(caps1)
    C0, C1 = int(caps0.sum()), int(caps1.sum())

    wT = np.ascontiguousarray(
        np.asarray(weights, dtype=np.float32).transpose(1, 0, 2)
        .reshape(IN, NUM_BANKS * OUT)).astype(BF16)
    bias_bf = np.asarray(bias, dtype=np.float32).astype(BF16)

    in_maps = []
    slotA_all = []
    for c in range(NCORES):
        toks = assign[c]
        sel = sel_all[toks]          # [NLOC, K]
        bw = bank_weights[toks]      # [NLOC, K]
        x_bf = tensor[toks].astype(BF16)   # [NLOC, IN]

        # slot maps: slot -> local token (or -1 pad), per pass
        slotA = np.full(C0, -1, dtype=np.int64)
        slotB = np.full(C1, -1, dtype=np.int64)
        a_of_tok = np.zeros(NLOC, dtype=np.int64)
        fill0, fill1 = offs0.copy(), offs1.copy()
        for i in range(NLOC):
            b0, b1 = sel[i, 0], sel[i, 1]
            s = fill0[b0]; fill0[b0] += 1
            slotA[s] = i; a_of_tok[i] = s
            s = fill1[b1]; fill1[b1] += 1
            slotB[s] = i

        validA, validB = slotA >= 0, slotB >= 0
        xa = np.zeros((C0, IN), dtype=BF16)
        xa[validA] = x_bf[slotA[validA]]
        xb = np.zeros((C1, IN), dtype=BF16)
        xb[validB] = x_bf[slotB[validB]]

        h01 = np.zeros((NUM_BANKS, C0 + C1), dtype=BF16)
        h01[sel[slotA[validA], 0], np.nonzero(validA)[0]] = 1
        h01[sel[slotB[validB], 1], C0 + np.nonzero(validB)[0]] = 1

        bwab = np.zeros(C0 + C1, dtype=np.float32)
        bwab[:C0][validA] = bw[slotA[validA], 0]
        bwab[C0:][validB] = bw[slotB[validB], 1]
        # column-major fold: slot j*128+p at [p, j]
        bwab = np.concatenate([
            bwab[:C0].reshape(C0 // P, P).T,
            bwab[C0:].reshape(C1 // P, P).T], axis=1)

        # scatter index: B slot -> A slot of same token; each pad slot adds
        # its zero row into a private dump row past C0 (shared dump rows race)
        sidx = np.zeros(C1, dtype=np.int64)
        sidx[validB] = a_of_tok[slotB[validB]]
        sidx[~validB] = C0 + np.arange(int((~validB).sum()))
        # wrap each 128-block group per scatter chunk of one psum tile
        chunks = []
        blk0 = 0
        for tw in [min(PSUM_FREE, C1 - t) for t in range(0, C1, PSUM_FREE)]:
            chunks.append(_wrap_idx(sidx[blk0:blk0 + tw]))
            blk0 += tw
        sidx_w = np.concatenate(chunks, axis=1)

        in_maps.append({
            "xa": xa,
            "xb": xb,
            "wts": wT,
            "biasb": bias_bf,
            "h01": h01,
            "bwab": np.ascontiguousarray(bwab),
            "sidx": sidx_w,
            "out": np.zeros((C0 + (C1 - NLOC), OUT), dtype=np.float32),
        })
        slotA_all.append(slotA)
    return in_maps, slotA_all


def kernel(tensor, bank_weights, bank_selections, weights, bias):
    tensor = np.asarray(tensor)
    bank_weights = np.asarray(bank_weights)
    bank_selections = np.asarray(bank_selections)
    weights = np.asarray(weights)
    bias = np.asarray(bias)

    assign, caps0, caps1 = _routing_plan(bank_selections)
    offs0, offs1 = _offsets(caps0), _offsets(caps1)
    C0, C1 = int(caps0.sum()), int(caps1.sum())
    segsA = _segments(caps0, offs0)
    segsB = _segments(caps1, offs1)
    nc = _build_program(C0, C1, segsA, segsB)
    in_maps, slotA_all = _make_in_maps(
        tensor, bank_weights, bank_selections, weights, bias,
        assign, caps0, caps1)

    nc.finalize()
    from concourse.bass_utils import run_bass_kernel_spmd
    try:
        res = run_bass_kernel_spmd(nc, in_maps, list(range(NCORES)))
    except Exception:
        # one retry: a previous crashed session can leave the accelerator in
        # a transient bad state that clears on the next dispatch
        import time
        time.sleep(2.0)
        res = run_bass_kernel_spmd(nc, in_maps, list(range(NCORES)))

    C0 = int(caps0.sum())
    out = np.empty((N, OUT), dtype=np.float32)
    for c in range(NCORES):
        ob = np.asarray(res.results[c]["out"])[:C0]
        slotA = slotA_all[c]
        m = slotA >= 0
        out[assign[c][slotA[m]]] = ob[m]
    return out


# revision 17
# speedup vs baseline: 1.1776x; 1.0243x over previous
"""BankedLinear (MoE-style banked linear) Trainium2 Bass kernel.

Math: out[n] = sum_k bank_weights[n,k] * (tensor[n] @ W[sel[n,k]] + bias[sel[n,k]])
Shapes: tensor [8192,128] f32, bank_weights [8192,2] f32, bank_selections [8192,2] int,
        weights [64,128,128] f32, bias [64,128] f32 -> out [8192,128] f32.

Strategy (data parallel over tokens, bf16 compute, two sorted passes):
  - 8 cores x 1024 tokens, greedy-balanced so per-bank per-pass counts are
    nearly equal across cores (SPMD: one program, shared bank capacities).
  - Pass A handles every token's k=0 pair sorted by sel0; pass B handles k=1
    sorted by sel1. Each pass:
      x slots loaded straight into transposed SBUF layout via dma_transpose
      (bf16), then per-bank matmuls W_b^T @ xT accumulate y^T into psum
      ([128 out, token cols], bf16 => 1 cycle/row). The bias term is seeded
      into the same psum by a rank-64 matmul bias^T @ H (H = 0/1 one-hot of
      the slot's bank, host built).
  - psum -> SBUF bf16 copy, PE transpose back to row layout, and the
    psum->SBUF copy of the transpose applies the per-token bank_weight as a
    tensor_scalar multiply (free).
  - Pass A rows are stored contiguously to the DRAM output; pass B rows are
    dma_scatter_add-ed (SWDGE, descriptors pre-generated via prepare_only)
    into the same buffer at the A-slot of the same token. Pad slots compute
    exact zeros and are pointed at row 0 (add of 0) or dropped by the host.
  - Host unpermutes: out[token_of_A_slot] = out_big[slot].
"""

import numpy as np
import ml_dtypes

N, K, IN, OUT, NUM_BANKS = 8192, 2, 128, 128, 64
NCORES = 8
NLOC = N // NCORES  # tokens per core
P = 128
PSUM_FREE = 512  # fp32 columns per psum bank
BF16 = ml_dtypes.bfloat16


def _routing_plan(sel_all):
    """Balance tokens across cores so that per-core per-bank counts of sel0
    and sel1 are close to the global ideal. Returns (assign [NCORES, NLOC],
    caps0, caps1) with caps shared by all cores (SPMD program)."""
    sel_all = np.asarray(sel_all).astype(np.int64)
    g0 = np.bincount(sel_all[:, 0], minlength=NUM_BANKS)
    g1 = np.bincount(sel_all[:, 1], minlength=NUM_BANKS)
    ideal0 = (g0 + NCORES - 1) // NCORES
    ideal1 = (g1 + NCORES - 1) // NCORES
    c0 = np.zeros((NCORES, NUM_BANKS), dtype=np.int64)
    c1 = np.zeros((NCORES, NUM_BANKS), dtype=np.int64)
    fill = np.zeros(NCORES, dtype=np.int64)
    assign_lists = [[] for _ in range(NCORES)]
    for n in range(N):
        b0, b1 = int(sel_all[n, 0]), int(sel_all[n, 1])
        best, best_key = -1, None
        for c in range(NCORES):
            if fill[c] >= NLOC:
                continue
            over = max(0, c0[c, b0] + 1 - ideal0[b0]) + max(
                0, c1[c, b1] + 1 - ideal1[b1])
            key = (over, c0[c, b0] + c1[c, b1], fill[c])
            if best < 0 or key < best_key:
                best, best_key = c, key
        c0[best, b0] += 1
        c1[best, b1] += 1
        fill[best] += 1
        assign_lists[best].append(n)
    assign = np.array(assign_lists, dtype=np.int64)

    caps0 = c0.max(axis=0).astype(np.int64)
    caps1 = c1.max(axis=0).astype(np.int64)
    # pad total slot counts to a multiple of 128 by growing the last bank
    caps0[NUM_BANKS - 1] += (-int(caps0.sum())) % P
    caps1[NUM_BANKS - 1] += (-int(caps1.sum())) % P
    return assign, caps0, caps1


def _offsets(caps):
    return np.concatenate([[0], np.cumsum(caps)[:-1]]).astype(np.int64)


def _segments(caps, offs):
    """Shared matmul segment list: (psum_tile, col0_in_tile, width, bank),
    bank ranges split at psum-tile (512 col) boundaries."""
    segs = []
    for b in range(NUM_BANKS):
        if caps[b] == 0:
            continue
        s0, s1 = int(offs[b]), int(offs[b] + caps[b])
        while s0 < s1:
            ti = s0 // PSUM_FREE
            e = min(s1, (ti + 1) * PSUM_FREE)
            segs.append((ti, s0 - ti * PSUM_FREE, e - s0, b))
            s0 = e
    return segs


def _wrap_idx(flat_idx):
    """Wrap a flat int16 index list into the [128, n//16] SWDGE layout:
    index i lives at [i % 16, i // 16], replicated across the 8 Q7 groups."""
    n = flat_idx.shape[0]
    assert n % 16 == 0
    w = flat_idx.reshape(n // 16, 16).T.astype(np.int16)
    return np.tile(w, (8, 1))


def _build_program(C0, C1, segsA, segsB):
    import concourse.bacc as bacc
    import concourse.tile as tile
    from concourse import mybir, library_config
    from concourse.masks import make_identity
    from concourse.tile import add_dep_helper

    f32 = mybir.dt.float32
    bf16 = mybir.dt.bfloat16
    i16 = mybir.dt.int16

    nbA, nbB = C0 // P, C1 // P  # 128-row blocks per pass
    tilesA = [min(PSUM_FREE, C0 - t) for t in range(0, C0, PSUM_FREE)]
    tilesB = [min(PSUM_FREE, C1 - t) for t in range(0, C1, PSUM_FREE)]
    assert len(tilesA) <= 3 and len(tilesB) <= 3, (C0, C1)

    nc = bacc.Bacc(None, target_bir_lowering=False, debug=False)

    xA_d = nc.declare_dram_parameter("xa", [C0, IN], bf16, isOutput=False)
    xB_d = nc.declare_dram_parameter("xb", [C1, IN], bf16, isOutput=False)
    w_d = nc.declare_dram_parameter("wts", [IN, NUM_BANKS * OUT], bf16,
                                    isOutput=False)
    bias_d = nc.declare_dram_parameter("biasb", [NUM_BANKS, OUT], bf16,
                                       isOutput=False)
    h_d = nc.declare_dram_parameter("h01", [NUM_BANKS, C0 + C1], bf16,
                                    isOutput=False)
    bw_d = nc.declare_dram_parameter("bwab", [P, (C0 + C1) // P], f32,
                                     isOutput=False)
    sidx_d = nc.declare_dram_parameter("sidx", [P, (C0 + C1) // 16], i16,
                                       isOutput=False)
    # extra dump rows: every pad slot of pass B scatter-adds into its own
    # private row (concurrent adds to a shared row race on real SWDGE)
    npad = C1 - NLOC
    out_d = nc.declare_dram_parameter("out", [C0 + npad, OUT], f32,
                                      isOutput=True)

    nblk_out = (C0 + (C1 - NLOC)) // P

    with tile.TileContext(nc) as tc:
        with (
            tc.tile_pool(name="const", bufs=1) as cpool,
            tc.tile_pool(name="big", bufs=1) as bigpool,
            tc.tile_pool(name="psum_a", bufs=1, space="PSUM") as psum_a,
            tc.tile_pool(name="psum_b", bufs=1, space="PSUM") as psum_b,
            tc.tile_pool(name="psum_t", bufs=2, space="PSUM") as psum_t,
        ):
            # SP ring: bias (tiny, gates seeds), xTA, W half 1, xTB
            bias_sb = cpool.tile([NUM_BANKS, OUT], bf16)
            nc.sync.dma_start(out=bias_sb[:], in_=bias_d.ap())
            xTA = bigpool.tile([P, C0], bf16, tag="xTA")
            nc.sync.dma_start_transpose(xTA[:], xA_d.ap())
            wT = bigpool.tile([P, NUM_BANKS * OUT], bf16, tag="wT")
            half = (NUM_BANKS // 2) * OUT
            nc.sync.dma_start(out=wT[:, :half], in_=w_d[:, :half])
            xTB = bigpool.tile([P, C1], bf16, tag="xTB")
            nc.sync.dma_start_transpose(xTB[:], xB_d.ap())

            # ACT ring: h01 (gates seeds), W half 2, then small tail inputs
            h_sb = cpool.tile([NUM_BANKS, C0 + C1], bf16)
            nc.scalar.dma_start(out=h_sb[:], in_=h_d.ap())
            nc.scalar.dma_start(out=wT[:, half:], in_=w_d[:, half:])
            sidx_sb = cpool.tile([P, (C0 + C1) // 16], i16)
            nc.scalar.dma_start(out=sidx_sb[:], in_=sidx_d.ap())
            bw_sb = cpool.tile([P, (C0 + C1) // P], f32)
            nc.scalar.dma_start(out=bw_sb[:], in_=bw_d.ap())
            # prime the ACT Copy LUT before the first real activation op
            warm = cpool.tile([P, 1], f32)
            nc.vector.memset(warm[:], 0.0)
            nc.scalar.activation(warm[:], warm[:],
                                 mybir.ActivationFunctionType.Copy)

            # Pool ring: identity, library, zero the output buffer (both
            # passes scatter-add into it with no cross-pass ordering)
            ident = cpool.tile([P, P], bf16)
            make_identity(nc, ident[:])
            libload = nc.gpsimd.load_library(library_config.mlp)
            ztile = bigpool.tile([P, nblk_out, OUT], f32, tag="ztile")
            nc.vector.memset(ztile[:], 0.0)
            zstore = nc.gpsimd.dma_start(
                out=out_d.ap().rearrange("(t p) o -> p t o", p=P),
                in_=ztile[:])

            # psum tiles per pass
            psA = [psum_a.tile([P, w], f32, tag=f"a{i}", name=f"psa{i}")
                   for i, w in enumerate(tilesA)]
            psB = [psum_b.tile([P, w], f32, tag=f"b{i}", name=f"psb{i}")
                   for i, w in enumerate(tilesB)]

            def run_pass(ps, tiles, segs, h_off, ti):
                """Per-segment bias seed + weight matmul for psum tile ti.
                Seed: psum[o, i] = sum_b bias[b, o] * H[b, i], then the
                bank's W matmul accumulates on top of it."""
                xT = xTA if h_off == 0 else xTB
                for (t, c0, cw, b) in segs:
                    if t != ti:
                        continue
                    g0 = t * PSUM_FREE + c0
                    nc.tensor.matmul(
                        out=ps[ti][:, c0:c0 + cw],
                        lhsT=bias_sb[:],
                        rhs=h_sb[:, h_off + g0:h_off + g0 + cw],
                        start=True, stop=False,
                    )
                    nc.tensor.matmul(
                        out=ps[ti][:, c0:c0 + cw],
                        lhsT=wT[:, b * OUT:(b + 1) * OUT],
                        rhs=xT[:, g0:g0 + cw],
                        start=False, stop=True,
                    )

            # W1-era tiles first, then W2-era (PE SEQ is in order)
            tile_order = []
            for i in range(max(len(tilesA), len(tilesB))):
                if i < len(tilesA):
                    tile_order.append(("A", i))
                if i < len(tilesB):
                    tile_order.append(("B", i))
            for (side, i) in tile_order:
                if side == "A":
                    run_pass(psA, tilesA, segsA, 0, i)
                else:
                    run_pass(psB, tilesB, segsB, C0, i)

            # psum^T -> SBUF bf16, PE transpose back, scaled copy to rows
            aT = bigpool.tile([P, C0], bf16, tag="aT")
            bT = bigpool.tile([P, C1], bf16, tag="bT")
            o0 = bigpool.tile([P, nbA, OUT], f32, tag="o0")
            o1 = bigpool.tile([P, nbB, OUT], f32, tag="o1")

            eng_i = 0

            def copy_eng():
                nonlocal eng_i
                eng_i += 1
                return nc.vector if eng_i % 2 == 0 else nc.scalar

            def finish_tile(side, ti):
                tw = (tilesA if side == "A" else tilesB)[ti]
                ps = (psA if side == "A" else psB)[ti]
                yT = aT if side == "A" else bT
                orows = o0 if side == "A" else o1
                bw_off = 0 if side == "A" else nbA
                col0 = ti * PSUM_FREE
                # big psum -> SBUF bf16 copy
                e = copy_eng()
                if e is nc.vector:
                    nc.vector.tensor_copy(yT[:, col0:col0 + tw], ps[:, :])
                else:
                    nc.scalar.copy(yT[:, col0:col0 + tw], ps[:, :])
                # per 128-col block: PE transpose back + scaled copy
                for j0 in range(0, tw, P):
                    j = (col0 + j0) // P
                    pt = psum_t.tile([P, P], bf16, tag="pt")
                    nc.tensor.transpose(out=pt[:], in_=yT[:, col0 + j0:
                                                          col0 + j0 + P],
                                        identity=ident[:])
                    e = copy_eng()
                    if e is nc.vector:
                        nc.vector.tensor_scalar_mul(
                            orows[:, j, :], pt[:], bw_sb[:, bw_off + j:
                                                         bw_off + j + 1])
                    else:
                        nc.scalar.activation(
                            orows[:, j, :], pt[:],
                            mybir.ActivationFunctionType.Copy,
                            scale=bw_sb[:, bw_off + j:bw_off + j + 1])

            for (side, i) in tile_order:
                finish_tile(side, i)

            # both passes scatter-add into the zeroed output buffer;
            # descriptors prepped early, trigger fires once data is ready
            for side, orows, nb, idx_off in (
                ("a", o0, nbA, 0), ("b", o1, nbB, nbA)):
                dma_sem = nc.alloc_semaphore(f"swdge_scat_{side}")
                prep = nc.gpsimd.dma_scatter_add(
                    out_d.ap(),
                    orows[:, :, :],
                    sidx_sb[:, idx_off * 8:(idx_off + nb) * 8],
                    nb * P, nb * P, OUT,
                    prepare_only=True, sem=dma_sem,
                )
                add_dep_helper(prep.ins, libload.ins, sync=False,
                               reason="scatter needs mlp gpsimd library")
                trig = nc.gpsimd.trigger_dma(count=None)
                add_dep_helper(trig.ins, zstore.ins, sync=True,
                               reason="scatter adds into zeroed buffer")

    return nc


def _make_in_maps(tensor, bank_weights, bank_selections, weights, bias,
                  assign, caps0, caps1):
    tensor = np.asarray(tensor, dtype=np.float32)
    bank_weights = np.asarray(bank_weights, dtype=np.float32)
    sel_all = np.asarray(bank_selections).astype(np.int64)
    offs0, offs1 = _offsets(caps0), _offsets(caps1)
    C0, C1 = int(caps0.sum()), int(caps1.sum())

    wT = np.ascontiguousarray(
        np.asarray(weights, dtype=np.float32).transpose(1, 0, 2)
        .reshape(IN, NUM_BANKS * OUT)).astype(BF16)
    bias_bf = np.asarray(bias, dtype=np.float32).astype(BF16)

    in_maps = []
    slotA_all = []
    for c in range(NCORES):
        toks = assign[c]
        sel = sel_all[toks]          # [NLOC, K]
        bw = bank_weights[toks]      # [NLOC, K]
        x_bf = tensor[toks].astype(BF16)   # [NLOC, IN]

        # slot maps: slot -> local token (or -1 pad), per pass
        slotA = np.full(C0, -1, dtype=np.int64)
        slotB = np.full(C1, -1, dtype=np.int64)
        a_of_tok = np.zeros(NLOC, dtype=np.int64)
        fill0, fill1 = offs0.copy(), offs1.copy()
        for i in range(NLOC):
            b0, b1 = sel[i, 0], sel[i, 1]
            s = fill0[b0]; fill0[b0] += 1
            slotA[s] = i; a_of_tok[i] = s
            s = fill1[b1]; fill1[b1] += 1
            slotB[s] = i

        validA, validB = slotA >= 0, slotB >= 0
        xa = np.zeros((C0, IN), dtype=BF16)
        xa[validA] = x_bf[slotA[validA]]
        xb = np.zeros((C1, IN), dtype=BF16)
        xb[validB] = x_bf[slotB[validB]]

        h01 = np.zeros((NUM_BANKS, C0 + C1), dtype=BF16)
        h01[sel[slotA[validA], 0], np.nonzero(validA)[0]] = 1
        h01[sel[slotB[validB], 1], C0 + np.nonzero(validB)[0]] = 1

        bwab = np.zeros(C0 + C1, dtype=np.float32)
        bwab[:C0][validA] = bw[slotA[validA], 0]
        bwab[C0:][validB] = bw[slotB[validB], 1]
        # column-major fold: slot j*128+p at [p, j]
        bwab = np.concatenate([
            bwab[:C0].reshape(C0 // P, P).T,
            bwab[C0:].reshape(C1 // P, P).T], axis=1)

        # scatter indices. Pass A: identity (row = own slot, pads add zero to
        # their own row). Pass B: A slot of the same token; each pad slot adds
        # its zero row into a private dump row past C0 (shared rows race).
        sidxB = np.zeros(C1, dtype=np.int64)
        sidxB[validB] = a_of_tok[slotB[validB]]
        sidxB[~validB] = C0 + np.arange(int((~validB).sum()))
        sidx_w = np.concatenate(
            [_wrap_idx(np.arange(C0, dtype=np.int64)), _wrap_idx(sidxB)],
            axis=1)

        in_maps.append({
            "xa": xa,
            "xb": xb,
            "wts": wT,
            "biasb": bias_bf,
            "h01": h01,
            "bwab": np.ascontiguousarray(bwab),
            "sidx": sidx_w,
            "out": np.zeros((C0 + (C1 - NLOC), OUT), dtype=np.float32),
        })
        slotA_all.append(slotA)
    return in_maps, slotA_all


def kernel(tensor, bank_weights, bank_selections, weights, bias):
    tensor = np.asarray(tensor)
    bank_weights = np.asarray(bank_weights)
    bank_selections = np.asarray(bank_selections)
    weights = np.asarray(weights)
    bias = np.asarray(bias)

    assign, caps0, caps1 = _routing_plan(bank_selections)
    offs0, offs1 = _offsets(caps0), _offsets(caps1)
    C0, C1 = int(caps0.sum()), int(caps1.sum())
    segsA = _segments(caps0, offs0)
    segsB = _segments(caps1, offs1)
    nc = _build_program(C0, C1, segsA, segsB)
    in_maps, slotA_all = _make_in_maps(
        tensor, bank_weights, bank_selections, weights, bias,
        assign, caps0, caps1)

    nc.finalize()
    from concourse.bass_utils import run_bass_kernel_spmd
    try:
        res = run_bass_kernel_spmd(nc, in_maps, list(range(NCORES)))
    except Exception:
        # one retry: a previous crashed session can leave the accelerator in
        # a transient bad state that clears on the next dispatch
        import time
        time.sleep(2.0)
        res = run_bass_kernel_spmd(nc, in_maps, list(range(NCORES)))

    C0 = int(caps0.sum())
    out = np.empty((N, OUT), dtype=np.float32)
    for c in range(NCORES):
        ob = np.asarray(res.results[c]["out"])[:C0]
        slotA = slotA_all[c]
        m = slotA >= 0
        out[assign[c][slotA[m]]] = ob[m]
    return out
